# revision 28
# baseline (speedup 1.0000x reference)
"""Trainium2 Bass kernel for nn_AdaptivePropagation (B=8 data-parallel,
one image per NeuronCore).

Per core:
  convs:  two 3x3 convs as K=96 (32ch x 3 dy-taps) matmuls streaming 4 rows
          x 128 cols (N=512) per instruction, 3 dx-taps accumulating in
          PSUM; 4 bands on the 4 PE column-groups. conv2 runs in bf16.
  sampling: exact bilinear via dense "hat" weights over a 7x7 shifted
          window at full 128-partition width, partition = (rr, band, k),
          bf16 MACs on per-partition replicated depth row-windows.
"""
import sys

sys.path.insert(0, "/opt/trn_rl_repo")

import numpy as np
import concourse.bass as bass
import concourse.tile as tile
import concourse.mybir as mybir
from concourse import bacc
from concourse.bass_utils import run_bass_kernel_spmd
from bass_rust import ScopedClock

fp32 = mybir.dt.float32
bf16 = mybir.dt.bfloat16
AL = mybir.AluOpType
AF = mybir.ActivationFunctionType

H, W, K, C = 512, 640, 8, 32
NB = 4            # bands
BR = H // NB      # rows per band (128)
WP = W + 4        # padded feat/h width (data at cols 1..640)
DWW = W + 8       # depth window width (data x at col 3+x)
NDS = 11          # depth window slots (rows y-3 .. y+7)
GR = 8            # rows per sampling group-iter
NIT = BR // GR    # sampling group iters (16)
NQ = BR // 4      # conv quads per band (32)
TAPS = list(range(-3, 4))
NCH = 5           # x chunks of 128


def _drain_and_barrier(self, tick_clock, wait_clock):
    # Tile's final drain can accumulate >1 sem wait; hw instructions hold
    # at most 1. Spread extras over SP nops.
    nc = self.nc
    drain_inst = nc.sync.drain()
    wait_clock.add_sem_waits(
        drain_inst.ins, ScopedClock({None: tick_clock.global_clock})
    )
    si = drain_inst.ins.sync_info
    waits = list(si.on_wait or []) if si is not None else []
    if len(waits) > 1:
        si.on_wait = waits[:1]
        drain_inst.ins.sync_info = si
        for w in waits[1:]:
            nop = nc.sync.nop(nofuse=True)
            nop.ins.sync_info = mybir.SyncInfo(on_wait=[w], on_update=[])
    nc.all_engine_barrier()
    assert self.sems is not None
    popped = nc._tile_sem_poison_stack.pop()
    assert popped is self._sem_poison
    nc.clear_and_free_semaphores(list(self.sems.allocated().values()))
    nc.all_engine_barrier()


tile.TileContext._drain_and_barrier = _drain_and_barrier


def _register_const(nc, val, dtype=fp32):
    if (dtype, val) in nc.const_aps.aps:
        return
    t = nc.alloc_sbuf_tensor(f"constx-{dtype.name}-{val}", [128, 1], dtype)
    nc.gpsimd.memset(t.ap(), val)
    nc.const_aps.aps[(dtype, val)] = t.ap()


def rawap(ap, off, dims):
    return bass.AP(ap.tensor, off, dims)


def build():
    nc = bacc.Bacc("TRN2", target_bir_lowering=False, debug=False, num_devices=8)
    for v in (-3.0, -2.0, -1.0, 2.0, 3.0):
        _register_const(nc, v)
    nc.all_engine_barrier()

    depth = nc.dram_tensor("depth", [H, W], fp32, kind="ExternalInput")
    feats = nc.dram_tensor("features", [C, H, W], fp32, kind="ExternalInput")
    w1 = nc.dram_tensor("w1", [32, 32, 3, 3], fp32, kind="ExternalInput")
    b1 = nc.dram_tensor("b1", [32], fp32, kind="ExternalInput")
    w2 = nc.dram_tensor("w2", [16, 32, 3, 3], fp32, kind="ExternalInput")
    b2 = nc.dram_tensor("b2", [16], fp32, kind="ExternalInput")
    out = nc.dram_tensor("out", [K + 1, H, W], fp32, kind="ExternalOutput")
    scr = nc.dram_tensor("scr_yb", [128], fp32)
    dbf = nc.dram_tensor("scr_dbf", [H, W], bf16)

    with tile.TileContext(nc) as tc:
        from contextlib import ExitStack
        ctx = ExitStack()
        cpool = ctx.enter_context(tc.tile_pool(name="consts", bufs=1))
        fbpool = ctx.enter_context(tc.tile_pool(name="fb", bufs=1))
        hqpool = ctx.enter_context(tc.tile_pool(name="hq", bufs=3))
        hbpool = ctx.enter_context(tc.tile_pool(name="hb", bufs=1))
        dwpool = ctx.enter_context(tc.tile_pool(name="dw", bufs=1))
        oxpool = ctx.enter_context(tc.tile_pool(name="oxy", bufs=2))
        wxpool = ctx.enter_context(tc.tile_pool(name="wx", bufs=1))
        stpool = ctx.enter_context(tc.tile_pool(name="stage", bufs=2))
        tmpool = ctx.enter_context(tc.tile_pool(name="tmp", bufs=1))
        ybpool = ctx.enter_context(tc.tile_pool(name="ybit", bufs=2))
        p1pool = ctx.enter_context(tc.tile_pool(name="ps1", bufs=1, space="PSUM"))
        p2pool = ctx.enter_context(tc.tile_pool(name="ps2", bufs=3, space="PSUM"))

        # ---- constant tiles -------------------------------------------
        # wt1k[(dy,ic), oc*3+dx] = w1[oc, ic, dy, dx]; wt2k likewise (bf16)
        wt1k = cpool.tile([96, 96], fp32, tag="wt1k")
        wt2kf = cpool.tile([96, 48], fp32, tag="wt2kf")
        wt2k = cpool.tile([96, 48], bf16, tag="wt2k")
        b1sb = cpool.tile([128, 1], fp32, tag="b1sb")
        b2sb = cpool.tile([128, 1], fp32, tag="b2sb")
        nc.vector.memset(b2sb[:], 0.0)
        for dy in range(3):
            nc.sync.dma_start(
                wt1k[32 * dy:32 * dy + 32, :],
                rawap(w1.ap(), dy * 3, [[9, 32], [288, 32], [1, 3]]))
            nc.sync.dma_start(
                wt2kf[32 * dy:32 * dy + 32, :],
                rawap(w2.ap(), dy * 3, [[9, 32], [288, 16], [1, 3]]))
        nc.vector.tensor_copy(wt2k[:], wt2kf[:])
        for b in range(NB):
            nc.sync.dma_start(
                b1sb[32 * b:32 * b + 32, :],
                rawap(b1.ap(), 0, [[1, 32], [1, 1]]))
            nc.sync.dma_start(
                b2sb[32 * b:32 * b + 16, :],
                rawap(b2.ap(), 0, [[1, 16], [1, 1]]))

        # YB0[p] = 128*b(p) + rr(p), p = 32rr+8b+k  (iota + DRAM bounce)
        t128 = cpool.tile([1, 128], fp32, tag="t128")
        nc.gpsimd.iota(t128[:].rearrange("p (a b c) -> p a b c", a=4, b=4, c=8),
                       [[1, 4], [128, 4], [0, 8]], base=0, channel_multiplier=0,
                       allow_small_or_imprecise_dtypes=True)
        nc.sync.dma_start(scr.ap(), t128[:])
        yb0 = cpool.tile([128, 1], fp32, tag="yb0")
        nc.sync.dma_start(yb0[:, 0:1], scr.ap().rearrange("(p x) -> p x", p=128))

        # x-iota tiles: NIX2 = -x, WIX2 = 639-x  (both [128, 2, W])
        nix1 = cpool.tile([128, W], fp32, tag="nix1")
        wix1 = cpool.tile([128, W], fp32, tag="wix1")
        nc.gpsimd.iota(nix1[:], [[-1, W]], base=0, channel_multiplier=0,
                       allow_small_or_imprecise_dtypes=True)
        nc.gpsimd.iota(wix1[:], [[-1, W]], base=W - 1, channel_multiplier=0,
                       allow_small_or_imprecise_dtypes=True)

        # one-time: bf16 depth copy in DRAM
        dstg = stpool.tile([128, 4, W], fp32, tag="osb4", name="dstg")
        dstgb = stpool.tile([128, 4, W], bf16, tag="osb4b", name="dstgb", bufs=1)
        nc.sync.dma_start(dstg[:],
                          rawap(depth.ap(), 0, [[W, 128], [128 * W, 4], [1, W]]))
        nc.vector.tensor_copy(dstgb[:], dstg[:])
        nc.sync.dma_start(rawap(dbf.ap(), 0, [[W, 128], [128 * W, 4], [1, W]]),
                          dstgb[:])



        # ---- conv helpers ---------------------------------------------
        def load_fb_band(q, b):
            fb = fbpool.tile([96, 4, WP], fp32, tag="fb", name="fb", bufs=4)
            nc.gpsimd.memset(fb[:, :, 0:1], 0.0)
            nc.gpsimd.memset(fb[:, :, 1 + W:WP], 0.0)
            for dy in range(3):
                g0 = BR * b + 4 * q - 1 + dy
                if 0 <= g0 and g0 + 3 < H:
                    nc.sync.dma_start(
                        fb[32 * dy:32 * dy + 32, :, 1:1 + W],
                        rawap(feats.ap(), g0 * W,
                              [[H * W, 32], [W, 4], [1, W]]))
                else:
                    for j in range(4):
                        g = g0 + j
                        if 0 <= g < H:
                            nc.sync.dma_start(
                                fb[32 * dy:32 * dy + 32, j, 1:1 + W],
                                rawap(feats.ap(), g * W,
                                      [[H * W, 32], [1, W]]))
                        else:
                            nc.gpsimd.memset(
                                fb[32 * dy:32 * dy + 32, j, :], 0.0)
            return fb

        def conv_q(fbs, wtk, n_oc, act_fn, bias, dst, dst_col0, dst_w,
                   band_major=False):
            """One conv layer on 4-row blocks for all bands/chunks; ACT
            writes into dst[128, 4, dst_w] at col offset dst_col0."""
            pool = p1pool if n_oc == 32 else p2pool
            if band_major:
                pss = [pool.tile([128, 512], fp32, tag=f"p1c{c}",
                                 name=f"p1c{c}") for c in range(NCH)]
                for b in range(NB):
                    fb = fbs(b)
                    for c in range(NCH):
                        for dx in range(3):
                            lhsT = wtk[:, dx:dx + 3 * (n_oc - 1) + 1:3]
                            rhs = rawap(fb[:], dx + 128 * c,
                                        [[4 * WP, 96], [WP, 4], [1, 128]])
                            nc.tensor.matmul(
                                pss[c][32 * b:32 * b + n_oc, :], lhsT, rhs,
                                start=(dx == 0), stop=(dx == 2),
                                tile_position=(0, 32 * b))
                for c in range(NCH):
                    dst_ap = rawap(dst[:], dst_col0 + 128 * c,
                                   [[4 * dst_w, 128], [dst_w, 4], [1, 128]])
                    psv = pss[c][:].rearrange("p (r x) -> p r x", r=4)
                    nc.scalar.activation(dst_ap, psv, act_fn, bias=bias[:])
                return
            for c in range(NCH):
                ps = pool.tile([128, 512], fp32, tag=f"ps{n_oc}",
                               name=f"ps{n_oc}")
                for b in range(NB):
                    for dx in range(3):
                        lhsT = wtk[:, dx:dx + 3 * (n_oc - 1) + 1:3]
                        rhs = rawap(fbs[b][:], dx + 128 * c,
                                    [[4 * WP, 96], [WP, 4], [1, 128]])
                        nc.tensor.matmul(
                            ps[32 * b:32 * b + n_oc, :], lhsT, rhs,
                            start=(dx == 0), stop=(dx == 2),
                            tile_position=(0, 32 * b))
                dst_ap = rawap(dst[:], dst_col0 + 128 * c,
                               [[4 * dst_w, 128], [dst_w, 4], [1, 128]])
                psv = ps[:].rearrange("p (r x) -> p r x", r=4)
                nc.scalar.activation(dst_ap, psv, act_fn, bias=bias[:])

        # extra h rows: hx[:, 0, :] = h(-1), hx[:, 1, :] = h(BR) per band
        hx = cpool.tile([128, 2, WP], bf16, tag="hx")
        nc.vector.memset(hx[:], 0.0)

        def conv1_single(local_row, hx_slot):
            pss = [p1pool.tile([128, 128], fp32, tag=f"p1c{c}",
                               name=f"p1cs{c}") for c in range(NCH)]
            for b in range(NB):
                fb = fbpool.tile([96, 4, WP], fp32, tag="fb", name="fbs",
                                 bufs=4)
                nc.gpsimd.memset(fb[:], 0.0)
                for dy in range(3):
                    g = BR * b + local_row - 1 + dy
                    if 0 <= g < H:
                        nc.sync.dma_start(
                            fb[32 * dy:32 * dy + 32, 0, 1:1 + W],
                            rawap(feats.ap(), g * W, [[H * W, 32], [1, W]]))
                for c in range(NCH):
                    for dx in range(3):
                        lhsT = wt1k[:, dx:dx + 3 * 31 + 1:3]
                        rhs = rawap(fb[:], dx + 128 * c,
                                    [[4 * WP, 96], [1, 128]])
                        nc.tensor.matmul(
                            pss[c][32 * b:32 * b + 32, :], lhsT, rhs,
                            start=(dx == 0), stop=(dx == 2),
                            tile_position=(0, 32 * b))
            for c in range(NCH):
                nc.scalar.activation(hx[:, hx_slot, 1 + 128 * c:129 + 128 * c],
                                     pss[c][:], AF.Relu, bias=b1sb[:])
            # conv2 SAME padding: h==0 outside the image
            if hx_slot == 0:
                nc.vector.memset(hx[0:32, 0:1, :], 0.0)
            else:
                nc.vector.memset(hx[96:128, 1:2, :], 0.0)

        def fill_hb(qp, hqs):
            """HB tiles for conv2 quad qp: group dy holds h rows
            4qp-1+dy .. 4qp+2+dy (band-local), from h-quad ring / hx."""
            hbs = []
            for b in range(NB):
                hb = hbpool.tile([96, 4, WP], bf16, tag=f"hb{b}",
                                 name=f"hb{b}")
                for dy in range(3):
                    r0 = 4 * qp - 1 + dy
                    j = 0
                    while j < 4:
                        r = r0 + j
                        if r < 0:
                            src = hx[32 * b:32 * b + 32, 0:1, :]
                            n = 1
                        elif r >= BR:
                            src = hx[32 * b:32 * b + 32, 1:2, :]
                            n = 1
                        else:
                            qq = r // 4
                            jj = r % 4
                            n = min(4 - j, 4 - jj)
                            src = hqs[qq][32 * b:32 * b + 32, jj:jj + n, :]
                        nc.scalar.dma_start(
                            hb[32 * dy:32 * dy + 32, j:j + n, :], src)
                        j += n
                hbs.append(hb)
            return hbs

        # ---- sampling --------------------------------------------------
        def sample_group(it, oxv, oyv, dwt):
            F2 = [128, 2, W]
            uc = tmpool.tile(F2, bf16, tag="uc")
            for y2 in range(2):
                nc.vector.tensor_tensor(uc[:, y2, :], oxv[:, y2, :], nix1[:],
                                        AL.max)
            nc.vector.tensor_tensor(uc[:], uc[:],
                                    rawap(wix1[:], 0, [[W, 128], [0, 2], [1, W]]),
                                    AL.min)
            vc = tmpool.tile(F2, bf16, tag="vc")
            for y2 in range(2):
                ylo = ybpool.tile([128, 1], fp32, tag="ylo")
                yhi = ybpool.tile([128, 1], fp32, tag="yhi")
                base = GR * it + 4 * y2
                nc.vector.tensor_scalar(ylo[:], yb0[:], -1.0, float(-base),
                                        AL.mult, AL.add)
                nc.vector.tensor_scalar(yhi[:], yb0[:], -1.0,
                                        float(H - 1 - base), AL.mult, AL.add)
                nc.vector.tensor_scalar(vc[:, y2, :], oyv[:, y2, :],
                                        ylo[:], yhi[:], AL.max, AL.min)

            wxn = {}
            for t in TAPS:
                ax = tmpool.tile(F2, bf16, tag="ax")
                nc.scalar.activation(ax[:], uc[:], AF.Abs, bias=float(-t))
                wt_ = wxpool.tile(F2, bf16, tag=f"wxn{t}", name=f"wxn{t}")
                nc.vector.tensor_scalar(wt_[:], ax[:], 1.0, 0.0,
                                        AL.subtract, AL.min)
                wxn[t] = wt_

            acc = tmpool.tile(F2, bf16, tag="acc")
            rowt = tmpool.tile(F2, bf16, tag="row")
            tmp = tmpool.tile(F2, bf16, tag="tmp")
            wyn = tmpool.tile(F2, bf16, tag="wyn")
            ay = tmpool.tile(F2, bf16, tag="ay")
            accf = tmpool.tile(F2, fp32, tag="accf")
            for si, s in enumerate(TAPS):
                nc.scalar.activation(ay[:], vc[:], AF.Abs, bias=float(-s))
                nc.vector.tensor_scalar(wyn[:], ay[:], 1.0, 0.0,
                                        AL.subtract, AL.min)
                for tj, t in enumerate(TAPS):
                    dv = rawap(dwt[:], (s + 3) * DWW + 3 + t,
                               [[NDS * DWW, 128], [4 * DWW, 2], [1, W]])
                    if tj == 0:
                        nc.vector.tensor_tensor(rowt[:], wxn[t][:], dv,
                                                AL.mult)
                    else:
                        nc.vector.tensor_tensor(tmp[:], wxn[t][:], dv,
                                                AL.mult)
                        nc.vector.tensor_tensor(rowt[:], rowt[:], tmp[:],
                                                AL.add)
                if si == 0:
                    nc.vector.tensor_tensor(acc[:], wyn[:], rowt[:], AL.mult)
                else:
                    nc.vector.tensor_tensor(tmp[:], wyn[:], rowt[:], AL.mult)
                    nc.vector.tensor_tensor(acc[:], acc[:], tmp[:], AL.add)

            nc.vector.tensor_copy(accf[:], acc[:])
            for rr in range(4):
                for y2 in range(2):
                    for b in range(NB):
                        off = (H * W + (BR * b + GR * it + 4 * y2 + rr) * W)
                        dst = rawap(out.ap(), off, [[H * W, 8], [1, W]])
                        nc.sync.dma_start(
                            dst, accf[32 * rr + 8 * b:32 * rr + 8 * b + 8,
                                      y2, :])

        def issue_dw(it):
            y0 = GR * it - 3
            edge = (it == 0 or it == NIT - 1)
            dst = dwt[:].rearrange("(rr b k) s x -> rr b k s x",
                                   rr=4, b=4, k=8)
            for b in range(NB):
                for rr in range(4):
                    if not edge:
                        src = rawap(dbf.ap(), (BR * b + y0 + rr) * W,
                                    [[0, 8], [W, NDS], [1, W]])
                        nc.sync.dma_start(dst[rr, b, :, :, 3:3 + W], src)
                    else:
                        g0 = BR * b + y0 + rr
                        jlo = max(0, -g0)
                        jhi = min(NDS, H - g0)
                        src = rawap(dbf.ap(), (g0 + jlo) * W,
                                    [[0, 8], [W, jhi - jlo], [1, W]])
                        nc.sync.dma_start(dst[rr, b, :, jlo:jhi, 3:3 + W],
                                          src)
                        for j in list(range(jlo)) + list(range(jhi, NDS)):
                            g = min(max(g0 + j, 0), H - 1)
                            src = rawap(dbf.ap(), g * W, [[0, 8], [1, W]])
                            nc.sync.dma_start(dst[rr, b, :, j, 3:3 + W], src)
            return dwt

        # ---- main loop -------------------------------------------------
        conv1_single(-1, 0)
        conv1_single(BR, 1)
        hq_tiles = [hqpool.tile([128, 4, WP], bf16, tag=f"hqt{j}",
                                name=f"hqt{j}", bufs=1) for j in range(3)]
        for t in hq_tiles:
            nc.vector.memset(t[:], 0.0)
        dwt = dwpool.tile([128, NDS, DWW], bf16, tag="dw")
        nc.vector.memset(dwt[:], 0.0)
        hq_ring = {}
        oxy = [None]
        dw_cur = [None]
        for q in range(NQ + 1):
            if q < NQ:
                hq = hq_tiles[q % 3]
                conv_q(lambda b, _q=q: load_fb_band(_q, b),
                       wt1k, 32, AF.Relu, b1sb, hq, 1, WP, band_major=True)
                hq_ring[q] = hq
            qp = q - 1
            if 0 <= qp < NQ:
                hbs = fill_hb(qp, hq_ring)
                osb4 = stpool.tile([128, 4, W], fp32, tag="osb4",
                                   name="osb4")
                conv_q(hbs, wt2k, 16, AF.Identity, b2sb, osb4, 0, W)
                y2 = qp % 2
                if y2 == 0:
                    oxy[0] = [oxpool.tile([128, 2, W], fp32, tag="ox",
                                          name="ox"),
                              oxpool.tile([128, 2, W], fp32, tag="oy",
                                          name="oy")]
                oxv, oyv = oxy[0]
                osv = osb4[:].rearrange("(b h k j) r x -> b h k j r x",
                                        b=4, h=2, k=8, j=2)
                for rr in range(4):
                    for b in range(NB):
                        nc.gpsimd.dma_start(
                            oxv[32 * rr + 8 * b:32 * rr + 8 * b + 8, y2, :],
                            osv[b, 0, :, 0, rr, :])
                        nc.gpsimd.dma_start(
                            oyv[32 * rr + 8 * b:32 * rr + 8 * b + 8, y2, :],
                            osv[b, 0, :, 1, rr, :])
                if qp % 2 == 0:
                    issue_dw(qp // 2)
                else:
                    sample_group(qp // 2, oxv, oyv, dwt)
                hq_ring.pop(qp - 1, None)

        # depth passthrough
        nc.sync.dma_start(rawap(out.ap(), 0, [[1, H * W]]),
                          rawap(depth.ap(), 0, [[1, H * W]]))
        ctx.close()
    nc.finalize()
    return nc


_CACHE = {}


def kernel(**inputs):
    if "nc" not in _CACHE:
        _CACHE["nc"] = build()
    nc = _CACHE["nc"]
    B = 8
    d = np.ascontiguousarray(np.asarray(inputs["depth"], np.float32))
    f = np.ascontiguousarray(np.asarray(inputs["features"], np.float32))
    w1v = np.ascontiguousarray(np.asarray(inputs["w1"], np.float32))
    b1v = np.ascontiguousarray(np.asarray(inputs["b1"], np.float32))
    w2v = np.ascontiguousarray(np.asarray(inputs["w2"], np.float32))
    b2v = np.ascontiguousarray(np.asarray(inputs["b2"], np.float32))
    in_maps = [{
        "depth": d[i, 0], "features": f[i],
        "w1": w1v, "b1": b1v, "w2": w2v, "b2": b2v,
    } for i in range(B)]
    res = run_bass_kernel_spmd(nc, in_maps, core_ids=list(range(B)))
    return np.stack([res.results[i]["out"] for i in range(B)], axis=0)


if __name__ == "__main__":
    data = np.load("/tmp/inputs.npz")
    outs = kernel(**{k: data[k] for k in data.files})
    exp = np.load("/tmp/expected.npy")
    rel = np.linalg.norm(outs - exp) / np.linalg.norm(exp)
    print("Relative error:", rel)


# revision 29
# speedup vs baseline: 1.1084x; 1.1084x over previous
"""Trainium2 Bass kernel for nn_AdaptivePropagation (B=8 data-parallel,
one image per NeuronCore).

Per core:
  convs:  two 3x3 convs as K=96 (32ch x 3 dy-taps) matmuls streaming 4 rows
          x 128 cols (N=512) per instruction, 3 dx-taps accumulating in
          PSUM; 4 bands on the 4 PE column-groups. conv2 runs in bf16.
  sampling: exact bilinear via dense "hat" weights over a 7x7 shifted
          window at full 128-partition width, partition = (rr, band, k),
          bf16 MACs on per-partition replicated depth row-windows.
"""
import sys

sys.path.insert(0, "/opt/trn_rl_repo")

import numpy as np
import concourse.bass as bass
import concourse.tile as tile
import concourse.mybir as mybir
from concourse import bacc
from concourse.bass_utils import run_bass_kernel_spmd
from bass_rust import ScopedClock

fp32 = mybir.dt.float32
bf16 = mybir.dt.bfloat16
AL = mybir.AluOpType
AF = mybir.ActivationFunctionType

H, W, K, C = 512, 640, 8, 32
NB = 4            # bands
BR = H // NB      # rows per band (128)
WP = W + 4        # padded feat/h width (data at cols 1..640)
DWW = W + 8       # depth window width (data x at col 3+x)
NDS = 11          # depth window slots (rows y-3 .. y+7)
GR = 8            # rows per sampling group-iter
NIT = BR // GR    # sampling group iters (16)
NQ = BR // 4      # conv quads per band (32)
TAPS = list(range(-3, 4))
NCH = 5           # x chunks of 128


def _drain_and_barrier(self, tick_clock, wait_clock):
    # Tile's final drain can accumulate >1 sem wait; hw instructions hold
    # at most 1. Spread extras over SP nops.
    nc = self.nc
    drain_inst = nc.sync.drain()
    wait_clock.add_sem_waits(
        drain_inst.ins, ScopedClock({None: tick_clock.global_clock})
    )
    si = drain_inst.ins.sync_info
    waits = list(si.on_wait or []) if si is not None else []
    if len(waits) > 1:
        si.on_wait = waits[:1]
        drain_inst.ins.sync_info = si
        for w in waits[1:]:
            nop = nc.sync.nop(nofuse=True)
            nop.ins.sync_info = mybir.SyncInfo(on_wait=[w], on_update=[])
    nc.all_engine_barrier()
    assert self.sems is not None
    popped = nc._tile_sem_poison_stack.pop()
    assert popped is self._sem_poison
    nc.clear_and_free_semaphores(list(self.sems.allocated().values()))
    nc.all_engine_barrier()


tile.TileContext._drain_and_barrier = _drain_and_barrier


def _register_const(nc, val, dtype=fp32):
    if (dtype, val) in nc.const_aps.aps:
        return
    t = nc.alloc_sbuf_tensor(f"constx-{dtype.name}-{val}", [128, 1], dtype)
    nc.gpsimd.memset(t.ap(), val)
    nc.const_aps.aps[(dtype, val)] = t.ap()


def rawap(ap, off, dims):
    return bass.AP(ap.tensor, off, dims)


def build():
    nc = bacc.Bacc("TRN2", target_bir_lowering=False, debug=False, num_devices=8)
    for v in (-3.0, -2.0, -1.0, 2.0, 3.0):
        _register_const(nc, v)
    nc.all_engine_barrier()

    depth = nc.dram_tensor("depth", [H, W], fp32, kind="ExternalInput")
    feats = nc.dram_tensor("features", [C, H, W], fp32, kind="ExternalInput")
    w1 = nc.dram_tensor("w1", [32, 32, 3, 3], fp32, kind="ExternalInput")
    b1 = nc.dram_tensor("b1", [32], fp32, kind="ExternalInput")
    w2 = nc.dram_tensor("w2", [16, 32, 3, 3], fp32, kind="ExternalInput")
    b2 = nc.dram_tensor("b2", [16], fp32, kind="ExternalInput")
    out = nc.dram_tensor("out", [K + 1, H, W], fp32, kind="ExternalOutput")
    scr = nc.dram_tensor("scr_yb", [128], fp32)
    dbf = nc.dram_tensor("scr_dbf", [H, W], bf16)

    with tile.TileContext(nc) as tc:
        from contextlib import ExitStack
        ctx = ExitStack()
        cpool = ctx.enter_context(tc.tile_pool(name="consts", bufs=1))
        fbpool = ctx.enter_context(tc.tile_pool(name="fb", bufs=1))
        hqpool = ctx.enter_context(tc.tile_pool(name="hq", bufs=3))
        hbpool = ctx.enter_context(tc.tile_pool(name="hb", bufs=1))
        dwpool = ctx.enter_context(tc.tile_pool(name="dw", bufs=1))
        oxpool = ctx.enter_context(tc.tile_pool(name="oxy", bufs=2))
        wxpool = ctx.enter_context(tc.tile_pool(name="wx", bufs=1))
        stpool = ctx.enter_context(tc.tile_pool(name="stage", bufs=2))
        tmpool = ctx.enter_context(tc.tile_pool(name="tmp", bufs=1))
        ybpool = ctx.enter_context(tc.tile_pool(name="ybit", bufs=2))
        p1pool = ctx.enter_context(tc.tile_pool(name="ps1", bufs=1, space="PSUM"))
        p2pool = ctx.enter_context(tc.tile_pool(name="ps2", bufs=3, space="PSUM"))

        # ---- constant tiles -------------------------------------------
        # wt1k[(dy,ic), oc*3+dx] = w1[oc, ic, dy, dx]; wt2k likewise (bf16)
        wt1kf = cpool.tile([96, 96], fp32, tag="wt1kf")
        wt1k = cpool.tile([96, 96], bf16, tag="wt1k")
        wt2kf = cpool.tile([96, 48], fp32, tag="wt2kf")
        wt2k = cpool.tile([96, 48], bf16, tag="wt2k")
        b1sb = cpool.tile([128, 1], fp32, tag="b1sb")
        b2sb = cpool.tile([128, 1], fp32, tag="b2sb")
        nc.vector.memset(b2sb[:], 0.0)
        for dy in range(3):
            nc.sync.dma_start(
                wt1kf[32 * dy:32 * dy + 32, :],
                rawap(w1.ap(), dy * 3, [[9, 32], [288, 32], [1, 3]]))
            nc.sync.dma_start(
                wt2kf[32 * dy:32 * dy + 32, :],
                rawap(w2.ap(), dy * 3, [[9, 32], [288, 16], [1, 3]]))
        nc.vector.tensor_copy(wt2k[:], wt2kf[:])
        nc.vector.tensor_copy(wt1k[:], wt1kf[:])
        for b in range(NB):
            nc.sync.dma_start(
                b1sb[32 * b:32 * b + 32, :],
                rawap(b1.ap(), 0, [[1, 32], [1, 1]]))
            nc.sync.dma_start(
                b2sb[32 * b:32 * b + 16, :],
                rawap(b2.ap(), 0, [[1, 16], [1, 1]]))

        # YB0[p] = 128*b(p) + rr(p), p = 32rr+8b+k  (iota + DRAM bounce)
        t128 = cpool.tile([1, 128], fp32, tag="t128")
        nc.gpsimd.iota(t128[:].rearrange("p (a b c) -> p a b c", a=4, b=4, c=8),
                       [[1, 4], [128, 4], [0, 8]], base=0, channel_multiplier=0,
                       allow_small_or_imprecise_dtypes=True)
        nc.sync.dma_start(scr.ap(), t128[:])
        yb0 = cpool.tile([128, 1], fp32, tag="yb0")
        nc.sync.dma_start(yb0[:, 0:1], scr.ap().rearrange("(p x) -> p x", p=128))

        # x-iota tiles: NIX2 = -x, WIX2 = 639-x  (both [128, 2, W])
        nix1 = cpool.tile([128, W], fp32, tag="nix1")
        wix1 = cpool.tile([128, W], fp32, tag="wix1")
        nc.gpsimd.iota(nix1[:], [[-1, W]], base=0, channel_multiplier=0,
                       allow_small_or_imprecise_dtypes=True)
        nc.gpsimd.iota(wix1[:], [[-1, W]], base=W - 1, channel_multiplier=0,
                       allow_small_or_imprecise_dtypes=True)

        # one-time: bf16 depth copy in DRAM
        dstg = stpool.tile([128, 4, W], fp32, tag="osb4", name="dstg")
        dstgb = stpool.tile([128, 4, W], bf16, tag="osb4b", name="dstgb", bufs=1)
        nc.sync.dma_start(dstg[:],
                          rawap(depth.ap(), 0, [[W, 128], [128 * W, 4], [1, W]]))
        nc.vector.tensor_copy(dstgb[:], dstg[:])
        nc.sync.dma_start(rawap(dbf.ap(), 0, [[W, 128], [128 * W, 4], [1, W]]),
                          dstgb[:])



        # ---- conv helpers ---------------------------------------------
        def load_fb_band(q, b):
            stg = fbpool.tile([96, 4, WP], fp32, tag="fbst", name="fbst",
                              bufs=2)
            nc.gpsimd.memset(stg[:, :, 0:1], 0.0)
            nc.gpsimd.memset(stg[:, :, 1 + W:WP], 0.0)
            for dy in range(3):
                g0 = BR * b + 4 * q - 1 + dy
                if 0 <= g0 and g0 + 3 < H:
                    nc.sync.dma_start(
                        stg[32 * dy:32 * dy + 32, :, 1:1 + W],
                        rawap(feats.ap(), g0 * W,
                              [[H * W, 32], [W, 4], [1, W]]))
                else:
                    for j in range(4):
                        g = g0 + j
                        if 0 <= g < H:
                            nc.sync.dma_start(
                                stg[32 * dy:32 * dy + 32, j, 1:1 + W],
                                rawap(feats.ap(), g * W,
                                      [[H * W, 32], [1, W]]))
                        else:
                            nc.gpsimd.memset(
                                stg[32 * dy:32 * dy + 32, j, :], 0.0)
            fb = fbpool.tile([96, 4, WP], bf16, tag="fb", name="fb", bufs=4)
            nc.scalar.activation(fb[:], stg[:], AF.Identity, bias=0.0)
            return fb

        def conv_q(fbs, wtk, n_oc, act_fn, bias, dst, dst_col0, dst_w,
                   band_major=False):
            """One conv layer on 4-row blocks for all bands/chunks; ACT
            writes into dst[128, 4, dst_w] at col offset dst_col0."""
            pool = p1pool if n_oc == 32 else p2pool
            if band_major:
                pss = [pool.tile([128, 512], fp32, tag=f"p1c{c}",
                                 name=f"p1c{c}") for c in range(NCH)]
                for b in range(NB):
                    fb = fbs(b)
                    for c in range(NCH):
                        for dx in range(3):
                            lhsT = wtk[:, dx:dx + 3 * (n_oc - 1) + 1:3]
                            rhs = rawap(fb[:], dx + 128 * c,
                                        [[4 * WP, 96], [WP, 4], [1, 128]])
                            nc.tensor.matmul(
                                pss[c][32 * b:32 * b + n_oc, :], lhsT, rhs,
                                start=(dx == 0), stop=(dx == 2),
                                tile_position=(0, 32 * b))
                for c in range(NCH):
                    dst_ap = rawap(dst[:], dst_col0 + 128 * c,
                                   [[4 * dst_w, 128], [dst_w, 4], [1, 128]])
                    psv = pss[c][:].rearrange("p (r x) -> p r x", r=4)
                    nc.scalar.activation(dst_ap, psv, act_fn, bias=bias[:])
                return
            for c in range(NCH):
                ps = pool.tile([128, 512], fp32, tag=f"ps{n_oc}",
                               name=f"ps{n_oc}")
                for b in range(NB):
                    for dx in range(3):
                        lhsT = wtk[:, dx:dx + 3 * (n_oc - 1) + 1:3]
                        rhs = rawap(fbs[b][:], dx + 128 * c,
                                    [[4 * WP, 96], [WP, 4], [1, 128]])
                        nc.tensor.matmul(
                            ps[32 * b:32 * b + n_oc, :], lhsT, rhs,
                            start=(dx == 0), stop=(dx == 2),
                            tile_position=(0, 32 * b))
                dst_ap = rawap(dst[:], dst_col0 + 128 * c,
                               [[4 * dst_w, 128], [dst_w, 4], [1, 128]])
                psv = ps[:].rearrange("p (r x) -> p r x", r=4)
                nc.scalar.activation(dst_ap, psv, act_fn, bias=bias[:])

        # extra h rows: hx[:, 0, :] = h(-1), hx[:, 1, :] = h(BR) per band
        hx = cpool.tile([128, 2, WP], bf16, tag="hx")
        nc.vector.memset(hx[:], 0.0)

        def conv1_single(local_row, hx_slot):
            pss = [p1pool.tile([128, 128], fp32, tag=f"p1c{c}",
                               name=f"p1cs{c}") for c in range(NCH)]
            for b in range(NB):
                stg = fbpool.tile([96, 4, WP], fp32, tag="fbst", name="stgs",
                                  bufs=2)
                nc.gpsimd.memset(stg[:], 0.0)
                for dy in range(3):
                    g = BR * b + local_row - 1 + dy
                    if 0 <= g < H:
                        nc.sync.dma_start(
                            stg[32 * dy:32 * dy + 32, 0, 1:1 + W],
                            rawap(feats.ap(), g * W, [[H * W, 32], [1, W]]))
                fb = fbpool.tile([96, 4, WP], bf16, tag="fb", name="fbs",
                                 bufs=4)
                nc.scalar.activation(fb[:], stg[:], AF.Identity, bias=0.0)
                for c in range(NCH):
                    for dx in range(3):
                        lhsT = wt1k[:, dx:dx + 3 * 31 + 1:3]
                        rhs = rawap(fb[:], dx + 128 * c,
                                    [[4 * WP, 96], [1, 128]])
                        nc.tensor.matmul(
                            pss[c][32 * b:32 * b + 32, :], lhsT, rhs,
                            start=(dx == 0), stop=(dx == 2),
                            tile_position=(0, 32 * b))
            for c in range(NCH):
                nc.scalar.activation(hx[:, hx_slot, 1 + 128 * c:129 + 128 * c],
                                     pss[c][:], AF.Relu, bias=b1sb[:])
            # conv2 SAME padding: h==0 outside the image
            if hx_slot == 0:
                nc.vector.memset(hx[0:32, 0:1, :], 0.0)
            else:
                nc.vector.memset(hx[96:128, 1:2, :], 0.0)

        def fill_hb(qp, hqs):
            """HB tiles for conv2 quad qp: group dy holds h rows
            4qp-1+dy .. 4qp+2+dy (band-local), from h-quad ring / hx."""
            hbs = []
            for b in range(NB):
                hb = hbpool.tile([96, 4, WP], bf16, tag=f"hb{b}",
                                 name=f"hb{b}")
                for dy in range(3):
                    r0 = 4 * qp - 1 + dy
                    j = 0
                    while j < 4:
                        r = r0 + j
                        if r < 0:
                            src = hx[32 * b:32 * b + 32, 0:1, :]
                            n = 1
                        elif r >= BR:
                            src = hx[32 * b:32 * b + 32, 1:2, :]
                            n = 1
                        else:
                            qq = r // 4
                            jj = r % 4
                            n = min(4 - j, 4 - jj)
                            src = hqs[qq][32 * b:32 * b + 32, jj:jj + n, :]
                        nc.scalar.dma_start(
                            hb[32 * dy:32 * dy + 32, j:j + n, :], src)
                        j += n
                hbs.append(hb)
            return hbs

        # ---- sampling --------------------------------------------------
        def sample_group(it, oxv, oyv, dwt):
            F2 = [128, 2, W]
            uc = tmpool.tile(F2, bf16, tag="uc")
            for y2 in range(2):
                nc.vector.tensor_tensor(uc[:, y2, :], oxv[:, y2, :], nix1[:],
                                        AL.max)
            nc.vector.tensor_tensor(uc[:], uc[:],
                                    rawap(wix1[:], 0, [[W, 128], [0, 2], [1, W]]),
                                    AL.min)
            vc = tmpool.tile(F2, bf16, tag="vc")
            for y2 in range(2):
                ylo = ybpool.tile([128, 1], fp32, tag="ylo")
                yhi = ybpool.tile([128, 1], fp32, tag="yhi")
                base = GR * it + 4 * y2
                nc.vector.tensor_scalar(ylo[:], yb0[:], -1.0, float(-base),
                                        AL.mult, AL.add)
                nc.vector.tensor_scalar(yhi[:], yb0[:], -1.0,
                                        float(H - 1 - base), AL.mult, AL.add)
                nc.vector.tensor_scalar(vc[:, y2, :], oyv[:, y2, :],
                                        ylo[:], yhi[:], AL.max, AL.min)

            wxn = {}
            for t in TAPS:
                ax = tmpool.tile(F2, bf16, tag="ax")
                nc.scalar.activation(ax[:], uc[:], AF.Abs, bias=float(-t))
                wt_ = wxpool.tile(F2, bf16, tag=f"wxn{t}", name=f"wxn{t}")
                nc.vector.tensor_scalar(wt_[:], ax[:], 1.0, 0.0,
                                        AL.subtract, AL.min)
                wxn[t] = wt_

            acc = tmpool.tile(F2, bf16, tag="acc")
            rowt = tmpool.tile(F2, bf16, tag="row")
            tmp = tmpool.tile(F2, bf16, tag="tmp")
            wyn = tmpool.tile(F2, bf16, tag="wyn")
            ay = tmpool.tile(F2, bf16, tag="ay")
            accf = tmpool.tile(F2, fp32, tag="accf")
            for si, s in enumerate(TAPS):
                nc.scalar.activation(ay[:], vc[:], AF.Abs, bias=float(-s))
                nc.vector.tensor_scalar(wyn[:], ay[:], 1.0, 0.0,
                                        AL.subtract, AL.min)
                for tj, t in enumerate(TAPS):
                    dv = rawap(dwt[:], (s + 3) * DWW + 3 + t,
                               [[NDS * DWW, 128], [4 * DWW, 2], [1, W]])
                    if tj == 0:
                        nc.vector.tensor_tensor(rowt[:], wxn[t][:], dv,
                                                AL.mult)
                    else:
                        nc.vector.tensor_tensor(tmp[:], wxn[t][:], dv,
                                                AL.mult)
                        nc.vector.tensor_tensor(rowt[:], rowt[:], tmp[:],
                                                AL.add)
                if si == 0:
                    nc.vector.tensor_tensor(acc[:], wyn[:], rowt[:], AL.mult)
                else:
                    nc.vector.tensor_tensor(tmp[:], wyn[:], rowt[:], AL.mult)
                    nc.vector.tensor_tensor(acc[:], acc[:], tmp[:], AL.add)

            nc.vector.tensor_copy(accf[:], acc[:])
            for rr in range(4):
                for y2 in range(2):
                    for b in range(NB):
                        off = (H * W + (BR * b + GR * it + 4 * y2 + rr) * W)
                        dst = rawap(out.ap(), off, [[H * W, 8], [1, W]])
                        nc.sync.dma_start(
                            dst, accf[32 * rr + 8 * b:32 * rr + 8 * b + 8,
                                      y2, :])

        def issue_dw(it):
            y0 = GR * it - 3
            edge = (it == 0 or it == NIT - 1)
            dst = dwt[:].rearrange("(rr b k) s x -> rr b k s x",
                                   rr=4, b=4, k=8)
            for b in range(NB):
                for rr in range(4):
                    if not edge:
                        src = rawap(dbf.ap(), (BR * b + y0 + rr) * W,
                                    [[0, 8], [W, NDS], [1, W]])
                        nc.sync.dma_start(dst[rr, b, :, :, 3:3 + W], src)
                    else:
                        g0 = BR * b + y0 + rr
                        jlo = max(0, -g0)
                        jhi = min(NDS, H - g0)
                        src = rawap(dbf.ap(), (g0 + jlo) * W,
                                    [[0, 8], [W, jhi - jlo], [1, W]])
                        nc.sync.dma_start(dst[rr, b, :, jlo:jhi, 3:3 + W],
                                          src)
                        for j in list(range(jlo)) + list(range(jhi, NDS)):
                            g = min(max(g0 + j, 0), H - 1)
                            src = rawap(dbf.ap(), g * W, [[0, 8], [1, W]])
                            nc.sync.dma_start(dst[rr, b, :, j, 3:3 + W], src)
            return dwt

        # ---- main loop -------------------------------------------------
        conv1_single(-1, 0)
        conv1_single(BR, 1)
        hq_tiles = [hqpool.tile([128, 4, WP], bf16, tag=f"hqt{j}",
                                name=f"hqt{j}", bufs=1) for j in range(3)]
        for t in hq_tiles:
            nc.vector.memset(t[:], 0.0)
        dwt = dwpool.tile([128, NDS, DWW], bf16, tag="dw")
        nc.vector.memset(dwt[:], 0.0)
        hq_ring = {}
        oxy = [None]
        dw_cur = [None]
        for q in range(NQ + 1):
            if q < NQ:
                hq = hq_tiles[q % 3]
                conv_q(lambda b, _q=q: load_fb_band(_q, b),
                       wt1k, 32, AF.Relu, b1sb, hq, 1, WP, band_major=True)
                hq_ring[q] = hq
            qp = q - 1
            if 0 <= qp < NQ:
                hbs = fill_hb(qp, hq_ring)
                osb4 = stpool.tile([128, 4, W], fp32, tag="osb4",
                                   name="osb4")
                conv_q(hbs, wt2k, 16, AF.Identity, b2sb, osb4, 0, W)
                y2 = qp % 2
                if y2 == 0:
                    oxy[0] = [oxpool.tile([128, 2, W], fp32, tag="ox",
                                          name="ox"),
                              oxpool.tile([128, 2, W], fp32, tag="oy",
                                          name="oy")]
                oxv, oyv = oxy[0]
                osv = osb4[:].rearrange("(b h k j) r x -> b h k j r x",
                                        b=4, h=2, k=8, j=2)
                for rr in range(4):
                    for b in range(NB):
                        nc.gpsimd.dma_start(
                            oxv[32 * rr + 8 * b:32 * rr + 8 * b + 8, y2, :],
                            osv[b, 0, :, 0, rr, :])
                        nc.gpsimd.dma_start(
                            oyv[32 * rr + 8 * b:32 * rr + 8 * b + 8, y2, :],
                            osv[b, 0, :, 1, rr, :])
                if qp % 2 == 0:
                    issue_dw(qp // 2)
                else:
                    sample_group(qp // 2, oxv, oyv, dwt)
                hq_ring.pop(qp - 1, None)

        # depth passthrough
        nc.sync.dma_start(rawap(out.ap(), 0, [[1, H * W]]),
                          rawap(depth.ap(), 0, [[1, H * W]]))
        ctx.close()
    nc.finalize()
    return nc


_CACHE = {}


def kernel(**inputs):
    if "nc" not in _CACHE:
        _CACHE["nc"] = build()
    nc = _CACHE["nc"]
    B = 8
    d = np.ascontiguousarray(np.asarray(inputs["depth"], np.float32))
    f = np.ascontiguousarray(np.asarray(inputs["features"], np.float32))
    w1v = np.ascontiguousarray(np.asarray(inputs["w1"], np.float32))
    b1v = np.ascontiguousarray(np.asarray(inputs["b1"], np.float32))
    w2v = np.ascontiguousarray(np.asarray(inputs["w2"], np.float32))
    b2v = np.ascontiguousarray(np.asarray(inputs["b2"], np.float32))
    in_maps = [{
        "depth": d[i, 0], "features": f[i],
        "w1": w1v, "b1": b1v, "w2": w2v, "b2": b2v,
    } for i in range(B)]
    res = run_bass_kernel_spmd(nc, in_maps, core_ids=list(range(B)))
    return np.stack([res.results[i]["out"] for i in range(B)], axis=0)


if __name__ == "__main__":
    data = np.load("/tmp/inputs.npz")
    outs = kernel(**{k: data[k] for k in data.files})
    exp = np.load("/tmp/expected.npy")
    rel = np.linalg.norm(outs - exp) / np.linalg.norm(exp)
    print("Relative error:", rel)


# revision 30
# speedup vs baseline: 1.1174x; 1.0080x over previous
"""Trainium2 Bass kernel for nn_AdaptivePropagation (B=8 data-parallel,
one image per NeuronCore).

Per core:
  convs:  two 3x3 convs as K=96 (32ch x 3 dy-taps) matmuls streaming 4 rows
          x 128 cols (N=512) per instruction, 3 dx-taps accumulating in
          PSUM; 4 bands on the 4 PE column-groups. conv2 runs in bf16.
  sampling: exact bilinear via dense "hat" weights over a 7x7 shifted
          window at full 128-partition width, partition = (rr, band, k),
          bf16 MACs on per-partition replicated depth row-windows.
"""
import sys

sys.path.insert(0, "/opt/trn_rl_repo")

import numpy as np
import concourse.bass as bass
import concourse.tile as tile
import concourse.mybir as mybir
from concourse import bacc
from concourse.bass_utils import run_bass_kernel_spmd
from bass_rust import ScopedClock

fp32 = mybir.dt.float32
bf16 = mybir.dt.bfloat16
AL = mybir.AluOpType
AF = mybir.ActivationFunctionType

H, W, K, C = 512, 640, 8, 32
NB = 4            # bands
BR = H // NB      # rows per band (128)
WP = W + 4        # padded feat/h width (data at cols 1..640)
DWW = W + 8       # depth window width (data x at col 3+x)
NDS = 11          # depth window slots (rows y-3 .. y+7)
GR = 8            # rows per sampling group-iter
NIT = BR // GR    # sampling group iters (16)
NQ = BR // 4      # conv quads per band (32)
TAPS = list(range(-3, 4))
NCH = 5           # x chunks of 128


def _drain_and_barrier(self, tick_clock, wait_clock):
    # Tile's final drain can accumulate >1 sem wait; hw instructions hold
    # at most 1. Spread extras over SP nops.
    nc = self.nc
    drain_inst = nc.sync.drain()
    wait_clock.add_sem_waits(
        drain_inst.ins, ScopedClock({None: tick_clock.global_clock})
    )
    si = drain_inst.ins.sync_info
    waits = list(si.on_wait or []) if si is not None else []
    if len(waits) > 1:
        si.on_wait = waits[:1]
        drain_inst.ins.sync_info = si
        for w in waits[1:]:
            nop = nc.sync.nop(nofuse=True)
            nop.ins.sync_info = mybir.SyncInfo(on_wait=[w], on_update=[])
    nc.all_engine_barrier()
    assert self.sems is not None
    popped = nc._tile_sem_poison_stack.pop()
    assert popped is self._sem_poison
    nc.clear_and_free_semaphores(list(self.sems.allocated().values()))
    nc.all_engine_barrier()


tile.TileContext._drain_and_barrier = _drain_and_barrier


def _register_const(nc, val, dtype=fp32):
    if (dtype, val) in nc.const_aps.aps:
        return
    t = nc.alloc_sbuf_tensor(f"constx-{dtype.name}-{val}", [128, 1], dtype)
    nc.gpsimd.memset(t.ap(), val)
    nc.const_aps.aps[(dtype, val)] = t.ap()


def rawap(ap, off, dims):
    return bass.AP(ap.tensor, off, dims)


def build():
    nc = bacc.Bacc("TRN2", target_bir_lowering=False, debug=False, num_devices=8)
    for v in (-3.0, -2.0, -1.0, 2.0, 3.0):
        _register_const(nc, v)
    nc.all_engine_barrier()

    depth = nc.dram_tensor("depth", [H, W], fp32, kind="ExternalInput")
    feats = nc.dram_tensor("features", [C, H, W], fp32, kind="ExternalInput")
    w1 = nc.dram_tensor("w1", [32, 32, 3, 3], fp32, kind="ExternalInput")
    b1 = nc.dram_tensor("b1", [32], fp32, kind="ExternalInput")
    w2 = nc.dram_tensor("w2", [16, 32, 3, 3], fp32, kind="ExternalInput")
    b2 = nc.dram_tensor("b2", [16], fp32, kind="ExternalInput")
    out = nc.dram_tensor("out", [K + 1, H, W], fp32, kind="ExternalOutput")
    scr = nc.dram_tensor("scr_yb", [128], fp32)
    dbf = nc.dram_tensor("scr_dbf", [H, W], bf16)

    with tile.TileContext(nc) as tc:
        from contextlib import ExitStack
        ctx = ExitStack()
        cpool = ctx.enter_context(tc.tile_pool(name="consts", bufs=1))
        fbpool = ctx.enter_context(tc.tile_pool(name="fb", bufs=1))
        hqpool = ctx.enter_context(tc.tile_pool(name="hq", bufs=3))
        hbpool = ctx.enter_context(tc.tile_pool(name="hb", bufs=1))
        dwpool = ctx.enter_context(tc.tile_pool(name="dw", bufs=1))
        oxpool = ctx.enter_context(tc.tile_pool(name="oxy", bufs=2))
        wxpool = ctx.enter_context(tc.tile_pool(name="wx", bufs=1))
        stpool = ctx.enter_context(tc.tile_pool(name="stage", bufs=2))
        tmpool = ctx.enter_context(tc.tile_pool(name="tmp", bufs=1))
        ybpool = ctx.enter_context(tc.tile_pool(name="ybit", bufs=2))
        p1pool = ctx.enter_context(tc.tile_pool(name="ps1", bufs=1, space="PSUM"))
        p2pool = ctx.enter_context(tc.tile_pool(name="ps2", bufs=3, space="PSUM"))

        # ---- constant tiles -------------------------------------------
        # wt1k[(dy,ic), oc*3+dx] = w1[oc, ic, dy, dx]; wt2k likewise (bf16)
        wt1kf = cpool.tile([96, 96], fp32, tag="wt1kf")
        wt1k = cpool.tile([96, 96], bf16, tag="wt1k")
        wt2kf = cpool.tile([96, 48], fp32, tag="wt2kf")
        wt2k = cpool.tile([96, 48], bf16, tag="wt2k")
        b1sb = cpool.tile([128, 1], fp32, tag="b1sb")
        b2sb = cpool.tile([128, 1], fp32, tag="b2sb")
        nc.vector.memset(b2sb[:], 0.0)
        for dy in range(3):
            nc.sync.dma_start(
                wt1kf[32 * dy:32 * dy + 32, :],
                rawap(w1.ap(), dy * 3, [[9, 32], [288, 32], [1, 3]]))
            nc.sync.dma_start(
                wt2kf[32 * dy:32 * dy + 32, :],
                rawap(w2.ap(), dy * 3, [[9, 32], [288, 16], [1, 3]]))
        nc.vector.tensor_copy(wt2k[:], wt2kf[:])
        nc.vector.tensor_copy(wt1k[:], wt1kf[:])
        for b in range(NB):
            nc.sync.dma_start(
                b1sb[32 * b:32 * b + 32, :],
                rawap(b1.ap(), 0, [[1, 32], [1, 1]]))
            nc.sync.dma_start(
                b2sb[32 * b:32 * b + 16, :],
                rawap(b2.ap(), 0, [[1, 16], [1, 1]]))

        # YB0[p] = 128*b(p) + rr(p), p = 32rr+8b+k  (iota + DRAM bounce)
        t128 = cpool.tile([1, 128], fp32, tag="t128")
        nc.gpsimd.iota(t128[:].rearrange("p (a b c) -> p a b c", a=4, b=4, c=8),
                       [[1, 4], [128, 4], [0, 8]], base=0, channel_multiplier=0,
                       allow_small_or_imprecise_dtypes=True)
        nc.sync.dma_start(scr.ap(), t128[:])
        yb0 = cpool.tile([128, 1], fp32, tag="yb0")
        nc.sync.dma_start(yb0[:, 0:1], scr.ap().rearrange("(p x) -> p x", p=128))

        # x-iota tiles: NIX2 = -x, WIX2 = 639-x  (both [128, 2, W])
        nix1 = cpool.tile([128, W], fp32, tag="nix1")
        wix1 = cpool.tile([128, W], fp32, tag="wix1")
        nc.gpsimd.iota(nix1[:], [[-1, W]], base=0, channel_multiplier=0,
                       allow_small_or_imprecise_dtypes=True)
        nc.gpsimd.iota(wix1[:], [[-1, W]], base=W - 1, channel_multiplier=0,
                       allow_small_or_imprecise_dtypes=True)

        # one-time: bf16 depth copy in DRAM
        dstg = stpool.tile([128, 4, W], fp32, tag="osb4", name="dstg")
        dstgb = stpool.tile([128, 4, W], bf16, tag="osb4b", name="dstgb", bufs=1)
        nc.sync.dma_start(dstg[:],
                          rawap(depth.ap(), 0, [[W, 128], [128 * W, 4], [1, W]]))
        nc.vector.tensor_copy(dstgb[:], dstg[:])
        nc.sync.dma_start(rawap(dbf.ap(), 0, [[W, 128], [128 * W, 4], [1, W]]),
                          dstgb[:])



        # ---- conv helpers ---------------------------------------------
        def load_fb_band(q, b):
            stg = fbpool.tile([96, 4, WP], fp32, tag="fbst", name="fbst",
                              bufs=2)
            nc.gpsimd.memset(stg[:, :, 0:1], 0.0)
            nc.gpsimd.memset(stg[:, :, 1 + W:WP], 0.0)
            for dy in range(3):
                g0 = BR * b + 4 * q - 1 + dy
                if 0 <= g0 and g0 + 3 < H:
                    nc.sync.dma_start(
                        stg[32 * dy:32 * dy + 32, :, 1:1 + W],
                        rawap(feats.ap(), g0 * W,
                              [[H * W, 32], [W, 4], [1, W]]))
                else:
                    for j in range(4):
                        g = g0 + j
                        if 0 <= g < H:
                            nc.sync.dma_start(
                                stg[32 * dy:32 * dy + 32, j, 1:1 + W],
                                rawap(feats.ap(), g * W,
                                      [[H * W, 32], [1, W]]))
                        else:
                            nc.gpsimd.memset(
                                stg[32 * dy:32 * dy + 32, j, :], 0.0)
            fb = fbpool.tile([96, 4, WP], bf16, tag="fb", name="fb", bufs=4)
            nc.scalar.activation(fb[:], stg[:], AF.Identity, bias=0.0)
            return fb

        def conv_q(fbs, wtk, n_oc, act_fn, bias, dst, dst_col0, dst_w,
                   band_major=False):
            """One conv layer on 4-row blocks for all bands/chunks; ACT
            writes into dst[128, 4, dst_w] at col offset dst_col0."""
            pool = p1pool if n_oc == 32 else p2pool
            if band_major:
                pss = [pool.tile([128, 512], fp32, tag=f"p1c{c}",
                                 name=f"p1c{c}") for c in range(NCH)]
                for b in range(NB):
                    fb = fbs(b)
                    for c in range(NCH):
                        for dx in range(3):
                            lhsT = wtk[:, dx:dx + 3 * (n_oc - 1) + 1:3]
                            rhs = rawap(fb[:], dx + 128 * c,
                                        [[4 * WP, 96], [WP, 4], [1, 128]])
                            nc.tensor.matmul(
                                pss[c][32 * b:32 * b + n_oc, :], lhsT, rhs,
                                start=(dx == 0), stop=(dx == 2),
                                tile_position=(0, 32 * b))
                for c in range(NCH):
                    dst_ap = rawap(dst[:], dst_col0 + 128 * c,
                                   [[4 * dst_w, 128], [dst_w, 4], [1, 128]])
                    psv = pss[c][:].rearrange("p (r x) -> p r x", r=4)
                    nc.scalar.activation(dst_ap, psv, act_fn, bias=bias[:])
                return
            for c in range(NCH):
                ps = pool.tile([128, 512], fp32, tag=f"ps{n_oc}",
                               name=f"ps{n_oc}")
                for b in range(NB):
                    for dx in range(3):
                        lhsT = wtk[:, dx:dx + 3 * (n_oc - 1) + 1:3]
                        rhs = rawap(fbs[b][:], dx + 128 * c,
                                    [[4 * WP, 96], [WP, 4], [1, 128]])
                        nc.tensor.matmul(
                            ps[32 * b:32 * b + n_oc, :], lhsT, rhs,
                            start=(dx == 0), stop=(dx == 2),
                            tile_position=(0, 32 * b))
                dst_ap = rawap(dst[:], dst_col0 + 128 * c,
                               [[4 * dst_w, 128], [dst_w, 4], [1, 128]])
                psv = ps[:].rearrange("p (r x) -> p r x", r=4)
                nc.scalar.activation(dst_ap, psv, act_fn, bias=bias[:])

        # extra h rows: hx[:, 0, :] = h(-1), hx[:, 1, :] = h(BR) per band
        hx = cpool.tile([128, 2, WP], bf16, tag="hx")
        nc.vector.memset(hx[:], 0.0)

        def conv1_single(local_row, hx_slot):
            pss = [p1pool.tile([128, 128], fp32, tag=f"p1c{c}",
                               name=f"p1cs{c}") for c in range(NCH)]
            for b in range(NB):
                stg = fbpool.tile([96, 4, WP], fp32, tag="fbst", name="stgs",
                                  bufs=2)
                nc.gpsimd.memset(stg[:], 0.0)
                for dy in range(3):
                    g = BR * b + local_row - 1 + dy
                    if 0 <= g < H:
                        nc.sync.dma_start(
                            stg[32 * dy:32 * dy + 32, 0, 1:1 + W],
                            rawap(feats.ap(), g * W, [[H * W, 32], [1, W]]))
                fb = fbpool.tile([96, 4, WP], bf16, tag="fb", name="fbs",
                                 bufs=4)
                nc.scalar.activation(fb[:], stg[:], AF.Identity, bias=0.0)
                for c in range(NCH):
                    for dx in range(3):
                        lhsT = wt1k[:, dx:dx + 3 * 31 + 1:3]
                        rhs = rawap(fb[:], dx + 128 * c,
                                    [[4 * WP, 96], [1, 128]])
                        nc.tensor.matmul(
                            pss[c][32 * b:32 * b + 32, :], lhsT, rhs,
                            start=(dx == 0), stop=(dx == 2),
                            tile_position=(0, 32 * b))
            for c in range(NCH):
                nc.scalar.activation(hx[:, hx_slot, 1 + 128 * c:129 + 128 * c],
                                     pss[c][:], AF.Relu, bias=b1sb[:])
            # conv2 SAME padding: h==0 outside the image
            if hx_slot == 0:
                nc.vector.memset(hx[0:32, 0:1, :], 0.0)
            else:
                nc.vector.memset(hx[96:128, 1:2, :], 0.0)

        def fill_hb(qp, hqs):
            """HB tiles for conv2 quad qp: group dy holds h rows
            4qp-1+dy .. 4qp+2+dy (band-local), from h-quad ring / hx."""
            hbs = []
            for b in range(NB):
                hb = hbpool.tile([96, 4, WP], bf16, tag=f"hb{b}",
                                 name=f"hb{b}")
                for dy in range(3):
                    r0 = 4 * qp - 1 + dy
                    j = 0
                    while j < 4:
                        r = r0 + j
                        if r < 0:
                            src = hx[32 * b:32 * b + 32, 0:1, :]
                            n = 1
                        elif r >= BR:
                            src = hx[32 * b:32 * b + 32, 1:2, :]
                            n = 1
                        else:
                            qq = r // 4
                            jj = r % 4
                            n = min(4 - j, 4 - jj)
                            src = hqs[qq][32 * b:32 * b + 32, jj:jj + n, :]
                        nc.scalar.dma_start(
                            hb[32 * dy:32 * dy + 32, j:j + n, :], src)
                        j += n
                hbs.append(hb)
            return hbs

        # ---- sampling --------------------------------------------------
        def sample_group(it, oxv, oyv, dwt):
            F2 = [128, 2, W]
            uc = tmpool.tile(F2, bf16, tag="uc")
            for y2 in range(2):
                nc.vector.tensor_tensor(uc[:, y2, :], oxv[:, y2, :], nix1[:],
                                        AL.max)
            nc.vector.tensor_tensor(uc[:], uc[:],
                                    rawap(wix1[:], 0, [[W, 128], [0, 2], [1, W]]),
                                    AL.min)
            vc = tmpool.tile(F2, bf16, tag="vc")
            for y2 in range(2):
                ylo = ybpool.tile([128, 1], fp32, tag="ylo")
                yhi = ybpool.tile([128, 1], fp32, tag="yhi")
                base = GR * it + 4 * y2
                nc.vector.tensor_scalar(ylo[:], yb0[:], -1.0, float(-base),
                                        AL.mult, AL.add)
                nc.vector.tensor_scalar(yhi[:], yb0[:], -1.0,
                                        float(H - 1 - base), AL.mult, AL.add)
                nc.vector.tensor_scalar(vc[:, y2, :], oyv[:, y2, :],
                                        ylo[:], yhi[:], AL.max, AL.min)

            wxn = {}
            for t in TAPS:
                ax = tmpool.tile(F2, bf16, tag="ax")
                nc.scalar.activation(ax[:], uc[:], AF.Abs, bias=float(-t))
                wt_ = wxpool.tile(F2, bf16, tag=f"wxn{t}", name=f"wxn{t}")
                nc.scalar.activation(wt_[:], ax[:], AF.Relu, bias=1.0,
                                     scale=-1.0)
                wxn[t] = wt_

            acc = tmpool.tile(F2, bf16, tag="acc")
            rowt = tmpool.tile(F2, bf16, tag="row")
            tmp = tmpool.tile(F2, bf16, tag="tmp")
            wyn = tmpool.tile(F2, bf16, tag="wyn")
            ay = tmpool.tile(F2, bf16, tag="ay")
            accf = tmpool.tile(F2, fp32, tag="accf")
            for si, s in enumerate(TAPS):
                nc.scalar.activation(ay[:], vc[:], AF.Abs, bias=float(-s))
                nc.scalar.activation(wyn[:], ay[:], AF.Relu, bias=1.0,
                                     scale=-1.0)
                for tj, t in enumerate(TAPS):
                    dv = rawap(dwt[:], (s + 3) * DWW + 3 + t,
                               [[NDS * DWW, 128], [4 * DWW, 2], [1, W]])
                    if tj == 0:
                        nc.vector.tensor_tensor(rowt[:], wxn[t][:], dv,
                                                AL.mult)
                    else:
                        nc.vector.tensor_tensor(tmp[:], wxn[t][:], dv,
                                                AL.mult)
                        nc.vector.tensor_tensor(rowt[:], rowt[:], tmp[:],
                                                AL.add)
                if si == 0:
                    nc.vector.tensor_tensor(acc[:], wyn[:], rowt[:], AL.mult)
                else:
                    nc.vector.tensor_tensor(tmp[:], wyn[:], rowt[:], AL.mult)
                    nc.vector.tensor_tensor(acc[:], acc[:], tmp[:], AL.add)

            nc.vector.tensor_copy(accf[:], acc[:])
            for rr in range(4):
                for y2 in range(2):
                    for b in range(NB):
                        off = (H * W + (BR * b + GR * it + 4 * y2 + rr) * W)
                        dst = rawap(out.ap(), off, [[H * W, 8], [1, W]])
                        nc.sync.dma_start(
                            dst, accf[32 * rr + 8 * b:32 * rr + 8 * b + 8,
                                      y2, :])

        def issue_dw(it):
            y0 = GR * it - 3
            edge = (it == 0 or it == NIT - 1)
            dst = dwt[:].rearrange("(rr b k) s x -> rr b k s x",
                                   rr=4, b=4, k=8)
            for b in range(NB):
                for rr in range(4):
                    if not edge:
                        src = rawap(dbf.ap(), (BR * b + y0 + rr) * W,
                                    [[0, 8], [W, NDS], [1, W]])
                        nc.sync.dma_start(dst[rr, b, :, :, 3:3 + W], src)
                    else:
                        g0 = BR * b + y0 + rr
                        jlo = max(0, -g0)
                        jhi = min(NDS, H - g0)
                        src = rawap(dbf.ap(), (g0 + jlo) * W,
                                    [[0, 8], [W, jhi - jlo], [1, W]])
                        nc.sync.dma_start(dst[rr, b, :, jlo:jhi, 3:3 + W],
                                          src)
                        for j in list(range(jlo)) + list(range(jhi, NDS)):
                            g = min(max(g0 + j, 0), H - 1)
                            src = rawap(dbf.ap(), g * W, [[0, 8], [1, W]])
                            nc.sync.dma_start(dst[rr, b, :, j, 3:3 + W], src)
            return dwt

        # ---- main loop -------------------------------------------------
        conv1_single(-1, 0)
        conv1_single(BR, 1)
        hq_tiles = [hqpool.tile([128, 4, WP], bf16, tag=f"hqt{j}",
                                name=f"hqt{j}", bufs=1) for j in range(3)]
        for t in hq_tiles:
            nc.vector.memset(t[:], 0.0)
        dwt = dwpool.tile([128, NDS, DWW], bf16, tag="dw")
        nc.vector.memset(dwt[:], 0.0)
        hq_ring = {}
        oxy = [None]
        dw_cur = [None]
        for q in range(NQ + 1):
            if q < NQ:
                hq = hq_tiles[q % 3]
                conv_q(lambda b, _q=q: load_fb_band(_q, b),
                       wt1k, 32, AF.Relu, b1sb, hq, 1, WP, band_major=True)
                hq_ring[q] = hq
            qp = q - 1
            if 0 <= qp < NQ:
                hbs = fill_hb(qp, hq_ring)
                osb4 = stpool.tile([128, 4, W], fp32, tag="osb4",
                                   name="osb4")
                conv_q(hbs, wt2k, 16, AF.Identity, b2sb, osb4, 0, W)
                y2 = qp % 2
                if y2 == 0:
                    oxy[0] = [oxpool.tile([128, 2, W], fp32, tag="ox",
                                          name="ox"),
                              oxpool.tile([128, 2, W], fp32, tag="oy",
                                          name="oy")]
                oxv, oyv = oxy[0]
                osv = osb4[:].rearrange("(b h k j) r x -> b h k j r x",
                                        b=4, h=2, k=8, j=2)
                for rr in range(4):
                    for b in range(NB):
                        nc.gpsimd.dma_start(
                            oxv[32 * rr + 8 * b:32 * rr + 8 * b + 8, y2, :],
                            osv[b, 0, :, 0, rr, :])
                        nc.gpsimd.dma_start(
                            oyv[32 * rr + 8 * b:32 * rr + 8 * b + 8, y2, :],
                            osv[b, 0, :, 1, rr, :])
                if qp % 2 == 0:
                    issue_dw(qp // 2)
                else:
                    sample_group(qp // 2, oxv, oyv, dwt)
                hq_ring.pop(qp - 1, None)

        # depth passthrough
        nc.sync.dma_start(rawap(out.ap(), 0, [[1, H * W]]),
                          rawap(depth.ap(), 0, [[1, H * W]]))
        ctx.close()
    nc.finalize()
    return nc


_CACHE = {}


def kernel(**inputs):
    if "nc" not in _CACHE:
        _CACHE["nc"] = build()
    nc = _CACHE["nc"]
    B = 8
    d = np.ascontiguousarray(np.asarray(inputs["depth"], np.float32))
    f = np.ascontiguousarray(np.asarray(inputs["features"], np.float32))
    w1v = np.ascontiguousarray(np.asarray(inputs["w1"], np.float32))
    b1v = np.ascontiguousarray(np.asarray(inputs["b1"], np.float32))
    w2v = np.ascontiguousarray(np.asarray(inputs["w2"], np.float32))
    b2v = np.ascontiguousarray(np.asarray(inputs["b2"], np.float32))
    in_maps = [{
        "depth": d[i, 0], "features": f[i],
        "w1": w1v, "b1": b1v, "w2": w2v, "b2": b2v,
    } for i in range(B)]
    res = run_bass_kernel_spmd(nc, in_maps, core_ids=list(range(B)))
    return np.stack([res.results[i]["out"] for i in range(B)], axis=0)


if __name__ == "__main__":
    data = np.load("/tmp/inputs.npz")
    outs = kernel(**{k: data[k] for k in data.files})
    exp = np.load("/tmp/expected.npy")
    rel = np.linalg.norm(outs - exp) / np.linalg.norm(exp)
    print("Relative error:", rel)
